# Initial kernel scaffold
#
"""Trainium2 Bass kernel for nn_Adapter_CrossNonParam (adapter + prompt/token cross-attention).

Data-parallel over batch: 8 NeuronCores x 4 batches each. Adapter weights are
replicated. All matmuls run in bf16 (fp32 PSUM accumulation); input x is
pre-transposed and cast to bf16 on the host so the device kernel needs no
layout shuffles for the big tensors.

Per-batch device pipeline (everything kept D-on-partition):
  downT[D,N] = W_down^T @ xT            (8 C-tile accumulation, PSUM)
  downT = gelu(downT + b_down)          (ScalarE, erf gelu, fused PSUM->SBUF, bf16)
  per token tile t (interleaved so ACT/DVE epilogue load spreads out):
    logitsT[t,P] = token_downT_t^T @ prompt_downT
    expT = exp(SCALE * logitsT)         (no max subtraction; logits are O(5))
    toktr[t] = PE-transpose of the token tile  ([tau, d] for the po matmul)
    token up-proj of tile t + PSUM->SBUF epilogue (alternating ACT/DVE)
  exp_acc[tau,P] = sum_t expT_t         (GpSimd tree, fp32 - spare engine)
  tail (deferred until after the NEXT batch's down matmuls, so the attention
  serial chain hides under PE-heavy work):
    denT[p,1] = exp_acc_half^T @ ones   (2 matmuls, partition-major denominator)
    recip = 1/denT                      (DVE)
    poT[D,P] += toktr_t^T @ expT_t      (accumulated, PSUM)
    prompt up-proj on UNNORMALIZED poT; normalization folded into the
    PSUM->SBUF epilogue as a per-partition scale.

DMA rings: x loads on the sync(SP) HWDGE ring; constants and output stores on
the scalar(ACT) HWDGE ring, so stores never block loads.
"""
import numpy as np
import ml_dtypes

import concourse.bass as bass
import concourse.tile as tile
from concourse import bacc, mybir
from concourse.bass_utils import run_bass_kernel_spmd

BF = mybir.dt.bfloat16
F32 = mybir.dt.float32

B, N, C = 32, 2248, 1024
D = 128
P = 200
T = N - P  # 2048
NCORES = 8
NB = B // NCORES  # 4 batches per core
SCALE = float(D) ** -0.5

CTILES = C // 128  # 8
TTILES = T // 128  # 16
DOWN_CHUNKS = [(s, min(512, N - s)) for s in range(0, N, 512)]  # 4x512 + 200


def build_nc():
    nc = bacc.Bacc("TRN2", target_bir_lowering=False, debug=False, num_devices=NCORES)

    xT = nc.dram_tensor("xT", [NB, C, N], BF, kind="ExternalInput")
    wdn = nc.dram_tensor("wdn", [128, CTILES, 128], BF, kind="ExternalInput")
    wup = nc.dram_tensor("wup", [D, C], BF, kind="ExternalInput")
    bdn = nc.dram_tensor("bdn", [D, 1], F32, kind="ExternalInput")
    ident = nc.dram_tensor("ident", [128, 128], BF, kind="ExternalInput")
    onesf = nc.dram_tensor("onesf", [128, 1], F32, kind="ExternalInput")
    out = nc.dram_tensor("out", [NB, N, C], BF, kind="ExternalOutput")

    with tile.TileContext(nc) as tc:
        with (
            tc.tile_pool(name="const", bufs=1) as const,
            tc.tile_pool(name="xp", bufs=3) as xp,
            tc.tile_pool(name="dg", bufs=4) as dg,
            tc.tile_pool(name="ex", bufs=2) as ex,
            tc.tile_pool(name="tt", bufs=2) as tt,
            tc.tile_pool(name="red", bufs=1) as red,
            tc.tile_pool(name="sm", bufs=1) as sm,
            tc.tile_pool(name="ob", bufs=6) as ob,
            # split rings: the down ring must not couple to the previous
            # batch's up epilogues. Transposes share the up ring (same tag).
            tc.tile_pool(name="ps_dn", bufs=2, space="PSUM") as ps_dn,
            tc.tile_pool(name="ps_up", bufs=4, space="PSUM") as ps_up,
            tc.tile_pool(name="ps_lg", bufs=1, space="PSUM") as ps_lg,
            tc.tile_pool(name="ps_po", bufs=1, space="PSUM") as ps_po,
        ):
            # ---- constants on the scalar ring (stores ring is empty at start,
            # so these never contend with the x loads on sync) ----
            wdn_sb = const.tile([128, CTILES, 128], BF)
            nc.scalar.dma_start(wdn_sb[:], wdn[:])
            bdn_sb = const.tile([D, 1], F32)
            nc.scalar.dma_start(bdn_sb[:], bdn[:])
            id_sb = const.tile([128, 128], BF)
            nc.scalar.dma_start(id_sb[:], ident[:])
            wup_sb = const.tile([D, C], BF)
            nc.scalar.dma_start(wup_sb[:], wup[:])
            onesf_sb = const.tile([128, 1], F32)
            nc.scalar.dma_start(onesf_sb[:], onesf[:])

            xsb_tiles = {}

            def load_x(b, split=False):
                """split=True -> 8 per-c-tile DMAs (fine-grained ramp; keeps
                all gelu chunks becoming ready together so the ACT table does
                not thrash between Gelu and Exp)."""
                xsb = xp.tile([128, CTILES, N], BF, tag="xsb")
                xsb_tiles[b] = xsb
                if split:
                    for n_piece, (s, w) in enumerate(
                        ((0, 512), (512, 512), (1024, 512), (1536, 712))
                    ):
                        src_ap = xT[b, :, s : s + w].rearrange(
                            "(a p) n -> p a n", p=128
                        )
                        nc.sync.dma_start(xsb[:, :, s : s + w], src_ap)
                else:
                    for h in range(2):
                        src = xT[b, h * 512 : (h + 1) * 512, :].rearrange(
                            "(a p) n -> p a n", p=128
                        )
                        nc.sync.dma_start(xsb[:, h * 4 : (h + 1) * 4, :], src)

            load_x(0, split=True)

            # token up-proj epilogue engine pattern: 5 DVE : 3 ACT per 8
            # halves (ACT also carries the exp chain; DVE the toktr copies
            # and exp partial sums)
            EPI_DVE = [True, False, True, True, False, True, True, False]
            half_ctr = [0]

            def up_tile_epilogue(dst, src):
                """PSUM->SBUF + cast, 5:3 DVE:ACT to balance engine load."""
                i = half_ctr[0] % 8
                half_ctr[0] += 1
                if EPI_DVE[i]:
                    nc.vector.tensor_copy(dst, src)
                else:
                    nc.scalar.copy(dst, src)

            def down(b):
                xsb = xsb_tiles[b]
                dng = dg.tile([128, N], BF, tag="dng")
                for s, w in DOWN_CHUNKS:
                    acc_full = ps_dn.tile([128, 512], F32, tag="dn")
                    acc = acc_full[:, :w]
                    for c in range(CTILES):
                        nc.tensor.matmul(
                            acc[:],
                            wdn_sb[:, c, :],
                            xsb[:, c, s : s + w],
                            start=(c == 0),
                            stop=(c == CTILES - 1),
                        )
                    nc.scalar.activation(
                        dng[:, s : s + w],
                        acc[:],
                        mybir.ActivationFunctionType.Gelu,
                        bias=bdn_sb[:],
                        scale=1.0,
                    )
                return dng

            def attn_and_token_up(b, dng):
                exps = ex.tile([128, TTILES, P], BF, tag="exps")
                toktr = tt.tile([128, TTILES, 128], BF, tag="toktr")
                # poT bank also hosts the partition-major denominator columns
                poT = ps_po.tile([128, P + 8], F32, tag="po")
                # two incremental fp32 accumulators on DVE (~155ns/tile rides
                # in the DVE slack of each tile-group; ready right after the
                # last exp instead of a bulk tree at the end)
                accA = red.tile([128, P], F32, tag="accA")
                accB = red.tile([128, P], F32, tag="accB")
                for j in range(TTILES // 2):
                    lg2 = ps_lg.tile([128, 2, P], F32, tag="lg")
                    osb = ob.tile([128, 2, C], BF, tag="osb")
                    for k in range(2):
                        t = 2 * j + k
                        tok = dng[:, P + t * 128 : P + (t + 1) * 128]
                        nc.tensor.matmul(
                            lg2[:, k, :], tok, dng[:, 0:P], start=True, stop=True
                        )
                        trp = ps_dn.tile([128, 128], BF, tag="dn")
                        nc.tensor.transpose(trp[:], tok, id_sb[:])
                        nc.vector.tensor_copy(toktr[:, t, :], trp[:])
                        for h in range(2):
                            up = ps_up.tile([128, 512], F32, tag="up")
                            nc.tensor.matmul(
                                up[:],
                                tok,
                                wup_sb[:, h * 512 : (h + 1) * 512],
                                start=True,
                                stop=True,
                            )
                            up_tile_epilogue(
                                osb[:, k, h * 512 : (h + 1) * 512], up[:]
                            )
                    # one exp over both logits tiles (they share a PSUM bank)
                    nc.scalar.activation(
                        exps[:, 2 * j : 2 * j + 2, :],
                        lg2[:],
                        mybir.ActivationFunctionType.Exp,
                        scale=SCALE,
                    )
                    if j == 0:
                        nc.vector.tensor_add(accA[:], exps[:, 0, :], exps[:, 1, :])
                    elif j == 1:
                        nc.vector.tensor_add(accB[:], exps[:, 2, :], exps[:, 3, :])
                    else:
                        nc.vector.tensor_add(accA[:], accA[:], exps[:, 2 * j, :])
                        nc.vector.tensor_add(accB[:], accB[:], exps[:, 2 * j + 1, :])
                    # po accumulation, pair-granular (waits only this pair's exp)
                    for k in range(2):
                        t = 2 * j + k
                        nc.tensor.matmul(
                            poT[:, 0:P],
                            toktr[:, t, :],
                            exps[:, t, :],
                            start=(t == 0),
                            stop=(t == TTILES - 1),
                        )
                    dstd = out[b, P + 256 * j : P + 256 * (j + 1), :].rearrange(
                        "(a p) c -> p a c", p=128
                    )
                    nc.gpsimd.dma_start(dstd, osb[:])
                return exps, toktr, poT, accA, accB

            def tail(b, dng, poT, accA, accB):
                """Attention tail of batch b. denT comes straight from the
                two partial accumulators (skips a final DVE add)."""
                nc.tensor.matmul(
                    poT[:, P : P + 1], accA[:, 0:128], onesf_sb[:],
                    start=True, stop=False,
                )
                nc.tensor.matmul(
                    poT[:, P : P + 1], accB[:, 0:128], onesf_sb[:],
                    start=False, stop=True,
                )
                nc.tensor.matmul(
                    poT[0:72, P + 1 : P + 2], accA[:, 128:200], onesf_sb[:],
                    start=True, stop=False,
                )
                nc.tensor.matmul(
                    poT[0:72, P + 1 : P + 2], accB[:, 128:200], onesf_sb[:],
                    start=False, stop=True,
                )
                rec0 = sm.tile([128, 1], F32, tag="rec0")
                nc.vector.reciprocal(rec0[:], poT[:, P : P + 1])
                rec1 = sm.tile([72, 1], F32, tag="rec1")
                nc.vector.reciprocal(rec1[:], poT[0:72, P + 1 : P + 2])
                # unnormalized prompt_out -> SBUF comb region (DVE; the ACT
                # queue must stay clear for the next batch's gelu)
                nc.vector.tensor_copy(dng[:, 0:P], poT[:, 0:P])

                # prompt up-proj; normalization via per-partition scale
                osbp = ob.tile([128, 2, C], BF, tag="osb")
                for h in range(2):
                    up = ps_up.tile([128, 512], F32, tag="up")
                    nc.tensor.matmul(
                        up[:],
                        dng[:, 0:128],
                        wup_sb[:, h * 512 : (h + 1) * 512],
                        start=True, stop=True,
                    )
                    nc.scalar.mul(osbp[:, 0, h * 512 : (h + 1) * 512], up[:], rec0[:])
                for h in range(2):
                    up = ps_up.tile([128, 512], F32, tag="up")
                    nc.tensor.matmul(
                        up[0:72, :],
                        dng[:, 128:200],
                        wup_sb[:, h * 512 : (h + 1) * 512],
                        start=True, stop=True,
                    )
                    nc.vector.tensor_scalar_mul(
                        osbp[0:72, 1, h * 512 : (h + 1) * 512], up[0:72, :], rec1[:]
                    )
                nc.gpsimd.dma_start(out[b, 0:128, :], osbp[:, 0, :])
                nc.gpsimd.dma_start(out[b, 128:200, :], osbp[0:72, 1, :])

            # phase 1: all down projections (ACT runs only Gelu -> one
            # table load); x loads pipeline on the sync ring
            dngs = {}
            for b in range(NB):
                if b + 1 < NB:
                    load_x(b + 1)
                dngs[b] = down(b)
            # phase 2: attention + up projection (ACT runs only Exp/Copy ->
            # one more table load); stores drain on the gpsimd ring
            for b in range(NB):
                exps, toktr, poT, accA, accB = attn_and_token_up(b, dngs[b])
                tail(b, dngs[b], poT, accA, accB)

    nc.compile()
    return nc


_NC_CACHE = None


def _get_nc():
    global _NC_CACHE
    if _NC_CACHE is None:
        _NC_CACHE = build_nc()
    return _NC_CACHE


def make_in_maps(x, W_down, b_down, W_up, b_up, gate):
    x = np.asarray(x, np.float32)
    W_down = np.asarray(W_down, np.float32)
    b_down = np.asarray(b_down, np.float32)
    W_up = np.asarray(W_up, np.float32)
    b_up = np.asarray(b_up, np.float32)
    gate = float(np.asarray(gate, np.float32))

    bf = ml_dtypes.bfloat16
    xT = np.ascontiguousarray(x.transpose(0, 2, 1)).astype(bf)  # [B, C, N]
    # wdn[p, c, m] = W_down[c*128 + p, m]
    wdn = np.ascontiguousarray(
        W_down.reshape(CTILES, 128, 128).transpose(1, 0, 2)
    ).astype(bf)
    wup = (W_up * gate).astype(bf)  # [D, C]
    bdn = b_down.reshape(D, 1).copy()
    ident = np.eye(128, dtype=bf)
    onesf = np.ones((128, 1), dtype=np.float32)

    in_maps = []
    for i in range(NCORES):
        in_maps.append(
            {
                "xT": np.ascontiguousarray(xT[i * NB : (i + 1) * NB]),
                "wdn": wdn,
                "wup": wup,
                "bdn": bdn,
                "ident": ident,
                "onesf": onesf,
            }
        )
    return in_maps


def kernel(**inputs):
    nc = _get_nc()
    in_maps = make_in_maps(**inputs)
    res = run_bass_kernel_spmd(nc, in_maps, core_ids=list(range(NCORES)))
    out = np.concatenate([res.results[i]["out"] for i in range(NCORES)], axis=0)
    out = out.astype(np.float32)
    # b_up (and gate) folded in on the host: device computes comb @ (gate*W_up)
    bias = (
        np.asarray(inputs["b_up"], np.float32)
        * float(np.asarray(inputs["gate"], np.float32))
    ).reshape(1, 1, C)
    return out + bias



# revision 1
# speedup vs baseline: 1.0282x; 1.0282x over previous
"""Trainium2 Bass kernel for nn_Adapter_CrossNonParam (adapter + prompt/token cross-attention).

Data-parallel over batch: 8 NeuronCores x 4 batches each. Adapter weights are
replicated. All matmuls run in bf16 (fp32 PSUM accumulation); input x is
pre-transposed and cast to bf16 on the host so the device kernel needs no
layout shuffles for the big tensors.

Per-batch device pipeline (everything kept D-on-partition):
  downT[D,N] = W_down^T @ xT            (8 C-tile accumulation, PSUM)
  downT = gelu(downT + b_down)          (ScalarE, erf gelu, fused PSUM->SBUF, bf16)
  per token tile t (interleaved so ACT/DVE epilogue load spreads out):
    logitsT[t,P] = token_downT_t^T @ prompt_downT
    expT = exp(SCALE * logitsT)         (no max subtraction; logits are O(5))
    toktr[t] = PE-transpose of the token tile  ([tau, d] for the po matmul)
    token up-proj of tile t + PSUM->SBUF epilogue (alternating ACT/DVE)
  exp_acc[tau,P] = sum_t expT_t         (GpSimd tree, fp32 - spare engine)
  tail (deferred until after the NEXT batch's down matmuls, so the attention
  serial chain hides under PE-heavy work):
    denT[p,1] = exp_acc_half^T @ ones   (2 matmuls, partition-major denominator)
    recip = 1/denT                      (DVE)
    poT[D,P] += toktr_t^T @ expT_t      (accumulated, PSUM)
    prompt up-proj on UNNORMALIZED poT; normalization folded into the
    PSUM->SBUF epilogue as a per-partition scale.

DMA rings: x loads on the sync(SP) HWDGE ring; constants and output stores on
the scalar(ACT) HWDGE ring, so stores never block loads.
"""
import numpy as np
import ml_dtypes

import concourse.bass as bass
import concourse.tile as tile
from concourse import bacc, mybir
from concourse.bass_utils import run_bass_kernel_spmd

BF = mybir.dt.bfloat16
F32 = mybir.dt.float32

B, N, C = 32, 2248, 1024
D = 128
P = 200
T = N - P  # 2048
NCORES = 8
NB = B // NCORES  # 4 batches per core
SCALE = float(D) ** -0.5

CTILES = C // 128  # 8
TTILES = T // 128  # 16
DOWN_CHUNKS = [(s, min(512, N - s)) for s in range(0, N, 512)]  # 4x512 + 200


def build_nc():
    nc = bacc.Bacc("TRN2", target_bir_lowering=False, debug=False, num_devices=NCORES)

    xT = nc.dram_tensor("xT", [NB, C, N], BF, kind="ExternalInput")
    wdn = nc.dram_tensor("wdn", [128, CTILES, 128], BF, kind="ExternalInput")
    wup = nc.dram_tensor("wup", [D, C], BF, kind="ExternalInput")
    bdn = nc.dram_tensor("bdn", [D, 1], F32, kind="ExternalInput")
    ident = nc.dram_tensor("ident", [128, 128], BF, kind="ExternalInput")
    onesf = nc.dram_tensor("onesf", [128, 1], F32, kind="ExternalInput")
    out = nc.dram_tensor("out", [NB, N, C], BF, kind="ExternalOutput")

    with tile.TileContext(nc) as tc:
        with (
            tc.tile_pool(name="const", bufs=1) as const,
            tc.tile_pool(name="xp", bufs=3) as xp,
            tc.tile_pool(name="dg", bufs=4) as dg,
            tc.tile_pool(name="ex", bufs=2) as ex,
            tc.tile_pool(name="tt", bufs=2) as tt,
            tc.tile_pool(name="red", bufs=1) as red,
            tc.tile_pool(name="sm", bufs=1) as sm,
            tc.tile_pool(name="ob", bufs=6) as ob,
            # split rings: the down ring must not couple to the previous
            # batch's up epilogues. Transposes share the up ring (same tag).
            tc.tile_pool(name="ps_dn", bufs=2, space="PSUM") as ps_dn,
            tc.tile_pool(name="ps_up", bufs=4, space="PSUM") as ps_up,
            tc.tile_pool(name="ps_lg", bufs=1, space="PSUM") as ps_lg,
            tc.tile_pool(name="ps_po", bufs=1, space="PSUM") as ps_po,
        ):
            # ---- constants on the scalar ring (stores ring is empty at start,
            # so these never contend with the x loads on sync) ----
            wdn_sb = const.tile([128, CTILES, 128], BF)
            nc.scalar.dma_start(wdn_sb[:], wdn[:])
            bdn_sb = const.tile([D, 1], F32)
            nc.scalar.dma_start(bdn_sb[:], bdn[:])
            id_sb = const.tile([128, 128], BF)
            nc.scalar.dma_start(id_sb[:], ident[:])
            wup_sb = const.tile([D, C], BF)
            nc.scalar.dma_start(wup_sb[:], wup[:])
            onesf_sb = const.tile([128, 1], F32)
            nc.scalar.dma_start(onesf_sb[:], onesf[:])

            xsb_tiles = {}

            def load_x(b, split=False):
                """split=True -> 8 per-c-tile DMAs (fine-grained ramp; keeps
                all gelu chunks becoming ready together so the ACT table does
                not thrash between Gelu and Exp)."""
                xsb = xp.tile([128, CTILES, N], BF, tag="xsb")
                xsb_tiles[b] = xsb
                if split:
                    for n_piece, (s, w) in enumerate(
                        ((0, 512), (512, 512), (1024, 512), (1536, 712))
                    ):
                        src_ap = xT[b, :, s : s + w].rearrange(
                            "(a p) n -> p a n", p=128
                        )
                        nc.sync.dma_start(xsb[:, :, s : s + w], src_ap)
                else:
                    for h in range(2):
                        src = xT[b, h * 512 : (h + 1) * 512, :].rearrange(
                            "(a p) n -> p a n", p=128
                        )
                        nc.sync.dma_start(xsb[:, h * 4 : (h + 1) * 4, :], src)

            load_x(0, split=True)

            # token up-proj epilogue engine pattern: 5 DVE : 3 ACT per 8
            # halves (ACT also carries the exp chain; DVE the toktr copies
            # and exp partial sums)
            EPI_DVE = [True, False, True, True, False, True, True, False]
            half_ctr = [0]

            def up_tile_epilogue(dst, src):
                """PSUM->SBUF + cast, 5:3 DVE:ACT to balance engine load."""
                i = half_ctr[0] % 8
                half_ctr[0] += 1
                if EPI_DVE[i]:
                    nc.vector.tensor_copy(dst, src)
                else:
                    nc.scalar.copy(dst, src)

            def down(b):
                xsb = xsb_tiles[b]
                dng = dg.tile([128, N], BF, tag="dng")
                for s, w in DOWN_CHUNKS:
                    acc_full = ps_dn.tile([128, 512], F32, tag="dn")
                    acc = acc_full[:, :w]
                    for c in range(CTILES):
                        nc.tensor.matmul(
                            acc[:],
                            wdn_sb[:, c, :],
                            xsb[:, c, s : s + w],
                            start=(c == 0),
                            stop=(c == CTILES - 1),
                        )
                    nc.scalar.activation(
                        dng[:, s : s + w],
                        acc[:],
                        mybir.ActivationFunctionType.Gelu,
                        bias=bdn_sb[:],
                        scale=1.0,
                    )
                return dng

            def attn_and_token_up(b, dng):
                exps = ex.tile([128, TTILES, P], BF, tag="exps")
                toktr = tt.tile([128, TTILES, 128], BF, tag="toktr")
                # poT bank also hosts the partition-major denominator columns
                poT = ps_po.tile([128, P + 8], F32, tag="po")
                # two incremental fp32 accumulators on DVE (~155ns/tile rides
                # in the DVE slack of each tile-group; ready right after the
                # last exp instead of a bulk tree at the end)
                accA = red.tile([128, P], F32, tag="accA")
                accB = red.tile([128, P], F32, tag="accB")
                for j in range(TTILES // 2):
                    lg2 = ps_lg.tile([128, 2, P], F32, tag="lg")
                    osb = ob.tile([128, 2, C], BF, tag="osb")
                    for k in range(2):
                        t = 2 * j + k
                        tok = dng[:, P + t * 128 : P + (t + 1) * 128]
                        nc.tensor.matmul(
                            lg2[:, k, :], tok, dng[:, 0:P], start=True, stop=True
                        )
                        trp = ps_dn.tile([128, 128], BF, tag="dn")
                        nc.tensor.transpose(trp[:], tok, id_sb[:])
                        nc.vector.tensor_copy(toktr[:, t, :], trp[:])
                        for h in range(2):
                            up = ps_up.tile([128, 512], F32, tag="up")
                            nc.tensor.matmul(
                                up[:],
                                tok,
                                wup_sb[:, h * 512 : (h + 1) * 512],
                                start=True,
                                stop=True,
                            )
                            up_tile_epilogue(
                                osb[:, k, h * 512 : (h + 1) * 512], up[:]
                            )
                    # one exp over both logits tiles (they share a PSUM bank)
                    nc.scalar.activation(
                        exps[:, 2 * j : 2 * j + 2, :],
                        lg2[:],
                        mybir.ActivationFunctionType.Exp,
                        scale=SCALE,
                    )
                    if j == 0:
                        nc.vector.tensor_add(accA[:], exps[:, 0, :], exps[:, 1, :])
                    elif j == 1:
                        nc.vector.tensor_add(accB[:], exps[:, 2, :], exps[:, 3, :])
                    else:
                        nc.vector.tensor_add(accA[:], accA[:], exps[:, 2 * j, :])
                        nc.vector.tensor_add(accB[:], accB[:], exps[:, 2 * j + 1, :])
                    # po accumulation, pair-granular (waits only this pair's exp)
                    for k in range(2):
                        t = 2 * j + k
                        nc.tensor.matmul(
                            poT[:, 0:P],
                            toktr[:, t, :],
                            exps[:, t, :],
                            start=(t == 0),
                            stop=(t == TTILES - 1),
                        )
                    dstd = out[b, P + 256 * j : P + 256 * (j + 1), :].rearrange(
                        "(a p) c -> p a c", p=128
                    )
                    nc.gpsimd.dma_start(dstd, osb[:])
                return exps, toktr, poT, accA, accB

            def tail(b, dng, poT, accA, accB):
                """Attention tail of batch b. denT comes straight from the
                two partial accumulators (skips a final DVE add)."""
                nc.tensor.matmul(
                    poT[:, P : P + 1], accA[:, 0:128], onesf_sb[:],
                    start=True, stop=False,
                )
                nc.tensor.matmul(
                    poT[:, P : P + 1], accB[:, 0:128], onesf_sb[:],
                    start=False, stop=True,
                )
                nc.tensor.matmul(
                    poT[0:72, P + 1 : P + 2], accA[:, 128:200], onesf_sb[:],
                    start=True, stop=False,
                )
                nc.tensor.matmul(
                    poT[0:72, P + 1 : P + 2], accB[:, 128:200], onesf_sb[:],
                    start=False, stop=True,
                )
                rec0 = sm.tile([128, 1], F32, tag="rec0")
                nc.vector.reciprocal(rec0[:], poT[:, P : P + 1])
                rec1 = sm.tile([72, 1], F32, tag="rec1")
                nc.vector.reciprocal(rec1[:], poT[0:72, P + 1 : P + 2])
                # unnormalized prompt_out -> SBUF comb region (DVE; the ACT
                # queue must stay clear for the next batch's gelu)
                nc.vector.tensor_copy(dng[:, 0:P], poT[:, 0:P])

                # prompt up-proj; normalization via per-partition scale
                osbp = ob.tile([128, 2, C], BF, tag="osb")
                for h in range(2):
                    up = ps_up.tile([128, 512], F32, tag="up")
                    nc.tensor.matmul(
                        up[:],
                        dng[:, 0:128],
                        wup_sb[:, h * 512 : (h + 1) * 512],
                        start=True, stop=True,
                    )
                    nc.scalar.mul(osbp[:, 0, h * 512 : (h + 1) * 512], up[:], rec0[:])
                for h in range(2):
                    up = ps_up.tile([128, 512], F32, tag="up")
                    nc.tensor.matmul(
                        up[0:72, :],
                        dng[:, 128:200],
                        wup_sb[:, h * 512 : (h + 1) * 512],
                        start=True, stop=True,
                    )
                    nc.vector.tensor_scalar_mul(
                        osbp[0:72, 1, h * 512 : (h + 1) * 512], up[0:72, :], rec1[:]
                    )
                nc.gpsimd.dma_start(out[b, 0:128, :], osbp[:, 0, :])
                nc.gpsimd.dma_start(out[b, 128:200, :], osbp[0:72, 1, :])

            # phase 1: all down projections (ACT runs only Gelu -> one
            # table load); x loads pipeline on the sync ring
            dngs = {}
            for b in range(NB):
                if b + 1 < NB:
                    load_x(b + 1)
                dngs[b] = down(b)
            # phase 2: attention + up projection (ACT runs only Exp/Copy ->
            # one more table load); stores drain on the gpsimd ring
            for b in range(NB):
                exps, toktr, poT, accA, accB = attn_and_token_up(b, dngs[b])
                tail(b, dngs[b], poT, accA, accB)

    nc.compile()
    return nc


_NC_CACHE = None


def _get_nc():
    global _NC_CACHE
    if _NC_CACHE is None:
        _NC_CACHE = build_nc()
    return _NC_CACHE


def make_in_maps(x, W_down, b_down, W_up, b_up, gate):
    x = np.asarray(x, np.float32)
    W_down = np.asarray(W_down, np.float32)
    b_down = np.asarray(b_down, np.float32)
    W_up = np.asarray(W_up, np.float32)
    b_up = np.asarray(b_up, np.float32)
    gate = float(np.asarray(gate, np.float32))

    bf = ml_dtypes.bfloat16
    xT = np.ascontiguousarray(x.transpose(0, 2, 1)).astype(bf)  # [B, C, N]
    # wdn[p, c, m] = W_down[c*128 + p, m]
    wdn = np.ascontiguousarray(
        W_down.reshape(CTILES, 128, 128).transpose(1, 0, 2)
    ).astype(bf)
    wup = (W_up * gate).astype(bf)  # [D, C]
    bdn = b_down.reshape(D, 1).copy()
    ident = np.eye(128, dtype=bf)
    onesf = np.ones((128, 1), dtype=np.float32)

    in_maps = []
    for i in range(NCORES):
        in_maps.append(
            {
                "xT": np.ascontiguousarray(xT[i * NB : (i + 1) * NB]),
                "wdn": wdn,
                "wup": wup,
                "bdn": bdn,
                "ident": ident,
                "onesf": onesf,
            }
        )
    return in_maps


def kernel(**inputs):
    nc = _get_nc()
    in_maps = make_in_maps(**inputs)
    res = run_bass_kernel_spmd(nc, in_maps, core_ids=list(range(NCORES)))
    out = np.concatenate([res.results[i]["out"] for i in range(NCORES)], axis=0)
    out = out.astype(np.float32)
    # b_up (and gate) folded in on the host: device computes comb @ (gate*W_up)
    bias = (
        np.asarray(inputs["b_up"], np.float32)
        * float(np.asarray(inputs["gate"], np.float32))
    ).reshape(1, 1, C)
    return out + bias

